# revision 1
# baseline (speedup 1.0000x reference)
"""Bass/Tile Trainium2 kernel for nn_AttentionSampling.

Problem: out = q + attention_downsampling(LN(q), LN(k), LN(v), factor=4)
  B=4, Sq=2048, Skv=8192, D=1024. Per query token s:
    w_f   = dot(LN(q)[s], LN(k)[4s+f])          f in 0..3  (no softmax)
    out[s] = q[s] + sum_f w_f * LN(v)[4s+f]

Key algebraic folding (valid for ln_weight==1, ln_bias==0, which is what
setup_inputs produces; a numpy fallback handles the general case):
    dot(LN(q), LN(k)) = aq*ak*(q.k - D*muq*muk)      a = rsqrt(var+eps)
    sum_f w_f*LN(v_f) = sum_f c_f*v_f - (sum_f c_f*muv_f)*ones,  c_f = w_f*av_f
so no normalized tensor is ever materialized: only raw dots + per-token stats.

Sharding: 8 cores = batch (4) x query-half (2). Each core owns 1024 windows:
q[1024,1024], k/v[1024,4,1024] (window-major view, 16KiB contiguous per
window), out[1024,1024]. 40 MiB HBM traffic per core => ~117us roofline.

Per 128-window tile, engine split:
  - DVE: bn_stats/bn_aggr for q and k_f, fused dot (tensor_tensor_reduce),
         small [128,4] weight math, final (q - d) + psum fuse.
  - ACT: v_f stats via activation(Copy/Square, accum_out=...)
  - PE : sum_f c_f*v_f as diag(c_f) @ v_f accumulated in PSUM.
"""

import numpy as np


def _ensure_concourse():
    try:
        import concourse.bass  # noqa: F401
    except ImportError:
        import sys

        for p in ("/opt/trn_rl_repo", "/root/.axon_site/_ro/trn_rl_repo"):
            if p not in sys.path:
                sys.path.insert(0, p)


_ensure_concourse()

import concourse.bass as bass  # noqa: E402
import concourse.tile as tile  # noqa: E402
from concourse import mybir  # noqa: E402
from concourse.bass_utils import run_bass_kernel_spmd  # noqa: E402

# ---------------------------------------------------------------------------
# Walrus-compatibility shims.
#
# The walrus in this container rejects two things Tile's end-of-context tail
# emits: (a) the final Drain carrying >2 sem waits ("Too many sync wait
# commands"), and (b) EVENT_SEMAPHORE_RANGE_CLEAR ("ISA wrong length").
# Replace the tail with per-semaphore EventSemaphore instructions that wait
# for each sem's final value and subtract it back to zero (equivalent:
# everything complete + sems cleared for re-execution), then the normal
# all-engine barrier. A JSON-level pass additionally splits any instruction
# carrying more than MAX_WAITS sem waits into EventSemaphore wait carriers.
# ---------------------------------------------------------------------------

_MAX_WAITS = 1


def _patched_drain_and_barrier(self, tick_clock, wait_clock):
    # NRT injects sema_reset into the NEFF pre/postamble (zeroing user sems
    # each execution), so no explicit sem clearing is needed. Just wait for
    # the async (DMA) sem increments to land, then barrier all engines.
    nc = self.nc
    gc = tick_clock.global_clock
    sems = self.sems.allocated()  # proc idx -> SemaphoreHandle
    for proc in sorted(sems):
        h = sems[proc]
        if "DMA" not in h.name:
            continue  # engine sems are implied by stream completion
        final = int(gc[proc]) * 16
        if final > 0:
            nc.gpsimd.wait_ge(h, final)
    nc.all_engine_barrier()
    popped = nc._tile_sem_poison_stack.pop()
    assert popped is self._sem_poison


tile.TileContext._drain_and_barrier = _patched_drain_and_barrier

_orig_to_json_bytes = bass.Bass.to_json_bytes


def _to_json_bytes_compat(self):
    import orjson

    raw = _orig_to_json_bytes(self)
    d = orjson.loads(raw)
    changed = False
    for fn in d.get("functions", []):
        blocks = fn.get("basic_blocks") or fn.get("blocks") or []
        for bb in blocks:
            insts = bb.get("instructions", [])
            new_insts = []
            for inst in insts:
                waits = (inst.get("sync_info") or {}).get("on_wait") or []
                if len(waits) > _MAX_WAITS:
                    keep = waits[-_MAX_WAITS:]
                    excess = waits[:-_MAX_WAITS]
                    for i, wt in enumerate(excess):
                        new_insts.append(
                            {
                                "name": f"{inst['name']}_wsplit{i}",
                                "opcode": "EventSemaphore",
                                "engine": inst["engine"],
                                "ins": [],
                                "outs": [],
                                "debug": inst.get("debug"),
                                "sync_info": {"on_update": [], "on_wait": [wt]},
                            }
                        )
                    inst["sync_info"]["on_wait"] = keep
                    changed = True
                new_insts.append(inst)
            bb["instructions"] = new_insts
    return orjson.dumps(d) if changed else raw


bass.Bass.to_json_bytes = _to_json_bytes_compat

F32 = mybir.dt.float32
ALU = mybir.AluOpType
ACTF = mybir.ActivationFunctionType
AXL = mybir.AxisListType

B, SQ, SKV, D = 4, 2048, 8192, 1024
FACTOR = 4
N_CORES = 8
W_PER_CORE = B * SQ // N_CORES  # 1024 windows per core
P = 128  # windows per tile = SBUF partitions
LN_EPS = 1e-5
HALF = 512  # PSUM bank free-dim (f32)


def build_bass(n_tiles=W_PER_CORE // P, repeats=1):
    """repeats>1 unrolls the whole tile loop N times (straight-line, no
    hardware loop) — used only for wall-clock exec-time measurement, since
    the axon NTFF profiling hook is unavailable in this image."""
    nc = bass.Bass()
    q_d = nc.declare_dram_parameter("q", [n_tiles * P, D], F32, isOutput=False)
    k_d = nc.declare_dram_parameter("k", [n_tiles * P, FACTOR, D], F32, isOutput=False)
    v_d = nc.declare_dram_parameter("v", [n_tiles * P, FACTOR, D], F32, isOutput=False)
    id_d = nc.declare_dram_parameter("ident", [P, P], F32, isOutput=False)
    o_d = nc.declare_dram_parameter("out", [n_tiles * P, D], F32, isOutput=True)

    with tile.TileContext(nc) as tc:
        with (
            tc.tile_pool(name="qp", bufs=4) as qp,
            tc.tile_pool(name="kp", bufs=4) as kp,
            tc.tile_pool(name="vp", bufs=4) as vp,
            tc.tile_pool(name="outp", bufs=3) as outp,
            tc.tile_pool(name="scratch", bufs=2) as scratch,
            tc.tile_pool(name="smalls", bufs=3) as sm,
            tc.tile_pool(name="const", bufs=1) as cp,
            tc.tile_pool(name="psum", bufs=4, space="PSUM") as pp,
        ):
            ident = cp.tile([P, P], F32)
            nc.sync.dma_start(ident[:], id_d[:])

            for _rep in range(repeats):
                for t in range(n_tiles):
                    rows = slice(t * P, (t + 1) * P)
                    q_sb = qp.tile([P, D], F32)
                    nc.sync.dma_start(q_sb[:], q_d[rows, :])
                    k_sb = kp.tile([P, FACTOR, D], F32)
                    nc.sync.dma_start(k_sb[:], k_d[rows, :, :])
                    # v goes through the SWDGE (gpsimd) ring so the two 2 MiB
                    # streams ride different DMA rings and overlap
                    v_sb = vp.tile([P, FACTOR, D], F32)
                    nc.gpsimd.dma_start(v_sb[:], v_d[rows, :, :])

                    # ---- stats: q, k_f on DVE (bn_stats), v_f on ACT (sum/sumsq)
                    # bn_stats is limited to 512 free elems per call -> 2 calls/tensor
                    bnst_q = sm.tile([P, 2, 6], F32)
                    for ch in range(2):
                        nc.vector.bn_stats(
                            bnst_q[:, ch], q_sb[:, ch * HALF : (ch + 1) * HALF]
                        )
                    aggr_q = sm.tile([P, 2], F32)  # (mean, var)
                    nc.vector.bn_aggr(aggr_q[:], bnst_q[:].rearrange("p c x -> p (c x)"))

                    bnst_k = sm.tile([P, FACTOR, 2, 6], F32)
                    aggr_k = sm.tile([P, FACTOR, 2], F32)
                    for f in range(FACTOR):
                        for ch in range(2):
                            nc.vector.bn_stats(
                                bnst_k[:, f, ch],
                                k_sb[:, f, ch * HALF : (ch + 1) * HALF],
                            )
                        nc.vector.bn_aggr(
                            aggr_k[:, f], bnst_k[:, f].rearrange("p c x -> p (c x)")
                        )

                    sums_v = sm.tile([P, FACTOR], F32)
                    ssq_v = sm.tile([P, FACTOR], F32)
                    for f in range(FACTOR):
                        dmp = scratch.tile([P, D], F32, tag="actdump")
                        nc.scalar.activation(
                            dmp[:], v_sb[:, f], ACTF.Copy, accum_out=sums_v[:, f : f + 1]
                        )
                        dmp2 = scratch.tile([P, D], F32, tag="actdump")
                        nc.scalar.activation(
                            dmp2[:], v_sb[:, f], ACTF.Square, accum_out=ssq_v[:, f : f + 1]
                        )

                    # mu_v = sums/D ; var_v = ssq/D - mu_v^2
                    mu_v = sm.tile([P, FACTOR], F32)
                    nc.gpsimd.tensor_scalar_mul(mu_v[:], sums_v[:], 1.0 / D)
                    mm_v = sm.tile([P, FACTOR], F32)
                    nc.gpsimd.tensor_mul(mm_v[:], mu_v[:], mu_v[:])
                    var_v = sm.tile([P, FACTOR], F32)
                    nc.vector.scalar_tensor_tensor(
                        var_v[:], ssq_v[:], 1.0 / D, mm_v[:], ALU.mult, ALU.subtract
                    )

                    # rstd = 1/sqrt(var+eps) via DVE reciprocal + ACT sqrt
                    tq = sm.tile([P, 1], F32)
                    nc.vector.tensor_scalar_add(tq[:], aggr_q[:, 1:2], LN_EPS)
                    rq = sm.tile([P, 1], F32)
                    nc.vector.reciprocal(rq[:], tq[:])
                    aq = sm.tile([P, 1], F32)
                    nc.scalar.sqrt(aq[:], rq[:])

                    tk = sm.tile([P, FACTOR], F32)
                    nc.vector.tensor_scalar_add(tk[:], aggr_k[:, :, 1], LN_EPS)
                    rk = sm.tile([P, FACTOR], F32)
                    nc.vector.reciprocal(rk[:], tk[:])
                    ak = sm.tile([P, FACTOR], F32)
                    nc.scalar.sqrt(ak[:], rk[:])

                    tv = sm.tile([P, FACTOR], F32)
                    nc.gpsimd.tensor_scalar_add(tv[:], var_v[:], LN_EPS)
                    rv = sm.tile([P, FACTOR], F32)
                    nc.vector.reciprocal(rv[:], tv[:])
                    av = sm.tile([P, FACTOR], F32)
                    nc.scalar.sqrt(av[:], rv[:])

                    # raw dots r_f = q . k_f  (fused multiply+accumulate via STT:
                    # out = (q bypass) * k_f, accum_out = sum(out); the raw-ISA
                    # tensor_tensor_reduce is rejected by this walrus)
                    rdots = sm.tile([P, FACTOR], F32)
                    for f in range(FACTOR):
                        prod = scratch.tile([P, D], F32, tag="prod")
                        nc.vector.scalar_tensor_tensor(
                            prod[:],
                            q_sb[:],
                            0.0,
                            k_sb[:, f],
                            ALU.bypass,
                            ALU.mult,
                            accum_out=rdots[:, f : f + 1],
                        )

                    # w_f = aq*ak_f*(r_f - D*muq*muk_f); c_f = w_f*av_f; d = sum c_f*muv_f
                    t1 = sm.tile([P, FACTOR], F32)
                    nc.gpsimd.tensor_scalar(
                        t1[:], aggr_k[:, :, 0], aggr_q[:, 0:1], None, ALU.mult
                    )
                    t2 = sm.tile([P, FACTOR], F32)
                    nc.vector.scalar_tensor_tensor(
                        t2[:], t1[:], -float(D), rdots[:], ALU.mult, ALU.add
                    )
                    u = sm.tile([P, FACTOR], F32)
                    nc.gpsimd.tensor_scalar(u[:], ak[:], aq[:, 0:1], None, ALU.mult)
                    w = sm.tile([P, FACTOR], F32)
                    nc.gpsimd.tensor_mul(w[:], t2[:], u[:])
                    c = sm.tile([P, FACTOR], F32)
                    nc.gpsimd.tensor_mul(c[:], w[:], av[:])
                    e = sm.tile([P, FACTOR], F32)
                    nc.gpsimd.tensor_mul(e[:], c[:], mu_v[:])
                    neg_d = sm.tile([P, 1], F32)
                    nc.vector.tensor_reduce(neg_d[:], e[:], AXL.X, ALU.add, negate=True)

                    # psum[s,:] = q[s,:] + sum_f c_f[s] * v_f[s,:]
                    # (q folded in as ident @ q; diag(c_f) built on idle GpSimd)
                    diags = []
                    for f in range(FACTOR):
                        dg = sm.tile([P, P], F32, tag=f"diag{f}")
                        nc.gpsimd.tensor_scalar_mul(dg[:], ident[:], c[:, f : f + 1])
                        diags.append(dg)
                    psum_t = pp.tile([P, 2, HALF], F32)
                    for h in range(2):
                        nc.tensor.matmul(
                            psum_t[:, h],
                            ident[:],
                            q_sb[:, h * HALF : (h + 1) * HALF],
                            start=True,
                            stop=False,
                        )
                        for f in range(FACTOR):
                            nc.tensor.matmul(
                                psum_t[:, h],
                                diags[f][:],
                                v_sb[:, f, h * HALF : (h + 1) * HALF],
                                start=False,
                                stop=(f == FACTOR - 1),
                            )

                    # out = psum - d   (ACT reads PSUM directly, per-token bias)
                    out_sb = outp.tile([P, D], F32)
                    nc.scalar.activation(
                        out_sb[:],
                        psum_t[:].rearrange("p c x -> p (c x)"),
                        ACTF.Identity,
                        bias=neg_d[:],
                    )
                    nc.sync.dma_start(o_d[rows, :], out_sb[:])
    return nc


_NC_CACHE = None


def _get_nc():
    global _NC_CACHE
    if _NC_CACHE is None:
        _NC_CACHE = build_bass()
    return _NC_CACHE


def _numpy_reference(query, key, value, ln_w, ln_b):
    def ln(x):
        mu = x.mean(-1, keepdims=True)
        var = ((x - mu) ** 2).mean(-1, keepdims=True)
        return (x - mu) / np.sqrt(var + LN_EPS) * ln_w + ln_b

    qn, kn, vn = ln(query), ln(key), ln(value)
    b, s, d = key.shape
    k_win = kn.reshape(b, s // FACTOR, FACTOR, d)
    wts = np.einsum("bsd,bsfd->bsf", qn, k_win).reshape(b, s)
    attn = (wts[:, :, None] * vn).reshape(b, s // FACTOR, FACTOR, d).sum(axis=2)
    return (query + attn).astype(np.float32)


def run(inputs, trace=False):
    """Returns (full_output, BassKernelResults-or-None)."""
    query = np.ascontiguousarray(np.asarray(inputs["query"], dtype=np.float32))
    key = np.ascontiguousarray(np.asarray(inputs["key"], dtype=np.float32))
    value = np.ascontiguousarray(np.asarray(inputs["value"], dtype=np.float32))
    ln_w = np.asarray(inputs["ln_weight"], dtype=np.float32)
    ln_b = np.asarray(inputs["ln_bias"], dtype=np.float32)

    if not (np.all(ln_w == 1.0) and np.all(ln_b == 0.0)):
        # General-path fallback (setup_inputs always produces ones/zeros).
        return _numpy_reference(query, key, value, ln_w, ln_b), None

    sq_h = SQ // 2  # 1024 query rows per core
    skv_h = SKV // 2  # 4096 kv rows per core
    ident = np.eye(P, dtype=np.float32)
    in_maps = []
    for cidx in range(N_CORES):
        bi, h = divmod(cidx, 2)
        qs = np.ascontiguousarray(query[bi, h * sq_h : (h + 1) * sq_h])
        ks = np.ascontiguousarray(key[bi, h * skv_h : (h + 1) * skv_h]).reshape(
            W_PER_CORE, FACTOR, D
        )
        vs = np.ascontiguousarray(value[bi, h * skv_h : (h + 1) * skv_h]).reshape(
            W_PER_CORE, FACTOR, D
        )
        in_maps.append({"q": qs, "k": ks, "v": vs, "ident": ident})

    res = run_bass_kernel_spmd(
        _get_nc(), in_maps, core_ids=list(range(N_CORES)), trace=trace
    )
    out = np.empty((B, SQ, D), dtype=np.float32)
    for cidx in range(N_CORES):
        bi, h = divmod(cidx, 2)
        out[bi, h * sq_h : (h + 1) * sq_h] = res.results[cidx]["out"]
    return out, res


def kernel(**inputs) -> np.ndarray:
    out, _ = run(inputs)
    return out

